# revision 15
# baseline (speedup 1.0000x reference)
"""CrossMatchingLoss Trainium2 kernel.

Problem: loss = -mean(matched cosine sims) where the matching is an exact
Hungarian assignment per batch element (detached / solved on CPU, exactly as
the reference does).

Split of work:
  host : L2-normalize (f32), transpose features to (D, N) layout, cast bf16,
         shard batches 4-per-core across 8 NeuronCores.
  device (per core): sim[b] = s_hat[b] @ t_hat[b]^T for its 4 batches as
         bf16 matmuls with f32 PSUM accumulation -> bf16 sim matrices.
  host : Hungarian assignment per batch on the device sim, then gather the
         matched cosine values (recomputed in f32/f64 for exactness) and
         average.

Shapes are hardcoded for B=32, N=256, D=1024, 8 cores (4 batches per core).
"""

import numpy as np
import ml_dtypes

B, N, D = 32, 256, 1024
N_CORES = 8
BPC = B // N_CORES          # batches per core
KC = D // 128               # contraction k-chunks of 128

_NC_CACHE = {}


def _build_bass():
    """Build (once) the per-core Bass program: 4x [256,1024]@[1024,256]^T."""
    if "nc" in _NC_CACHE:
        return _NC_CACHE["nc"]

    import concourse.bacc as bacc
    import concourse.mybir as mybir
    import concourse.tile as tile

    nc = bacc.Bacc("TRN2")
    # p-major packing split in two k-halves: x[b, h, p, st, kk, :] =
    # feat_st[b, (h*4 + kk)*128 + p, :]. Each partition's data per DMA is one
    # contiguous 4KB run; the two halves load on different HWDGE rings.
    KH = KC // 2  # 4 k-chunks per half
    x = nc.dram_tensor(
        "x", [BPC, 2, 128, 2, KH, N], mybir.dt.bfloat16, kind="ExternalInput"
    )
    sim = nc.dram_tensor(
        "sim", [BPC, 128, 2, N], mybir.dt.bfloat16, kind="ExternalOutput"
    )

    with tile.TileContext(nc) as tc:
        with (
            tc.tile_pool(name="xin", bufs=4) as xin_pool,
            tc.tile_pool(name="so", bufs=1) as so_pool,
            tc.tile_pool(name="ps", bufs=2, space="PSUM") as ps_pool,
        ):
            # dummy matmuls during the initial DMA wait: get the PE past the
            # HAM cold window (~3.4us) so real matmuls run at 2.4 GHz
            zt = xin_pool.tile([128, 128], mybir.dt.bfloat16, name="zt", tag="zt")
            nc.gpsimd.memset(zt[:], 0.0)
            ps_w = ps_pool.tile(
                [128, 128], mybir.dt.float32, name="ps_w", tag="warm"
            )
            for _ in range(20):
                nc.tensor.matmul(ps_w[:], zt[:], zt[:], start=True, stop=True)

            so_tiles = []
            for b in range(BPC):
                xh0 = xin_pool.tile(
                    [128, 2, KH, N], mybir.dt.bfloat16, name=f"xh0_{b}", tag="h0"
                )
                # last batch: student half goes on the scalar ring and the
                # teacher quarters on sync, so both rings carry 2MB and the
                # final chunks drain as early as possible
                (nc.sync if b < BPC - 1 else nc.scalar).dma_start(
                    out=xh0[:], in_=x[b, 0]
                )
                if b < BPC - 1:
                    xh1 = xin_pool.tile(
                        [128, 2, KH, N], mybir.dt.bfloat16, name=f"xh1_{b}",
                        tag="h1",
                    )
                    nc.scalar.dma_start(out=xh1[:], in_=x[b, 1])

                    def chunk(k, xh0=xh0, xh1=xh1):
                        return (xh0, k) if k < KH else (xh1, k - KH)
                else:
                    # last batch: teacher half as two quarter loads so only a
                    # couple of matmuls trail the final chunk's arrival
                    xq0 = xin_pool.tile(
                        [128, 2, 2, N], mybir.dt.bfloat16, name="xq0", tag="q0"
                    )
                    xq1 = xin_pool.tile(
                        [128, 2, 2, N], mybir.dt.bfloat16, name="xq1", tag="q1"
                    )
                    nc.sync.dma_start(out=xq0[:], in_=x[b, 1][:, :, 0:2, :])
                    nc.sync.dma_start(out=xq1[:], in_=x[b, 1][:, :, 2:4, :])

                    def chunk(k, xh0=xh0, xq0=xq0, xq1=xq1):
                        if k < KH:
                            return (xh0, k)
                        if k < KH + 2:
                            return (xq0, k - KH)
                        return (xq1, k - KH - 2)

                for j in range(2):
                    ps = ps_pool.tile(
                        [128, N], mybir.dt.float32, name=f"ps_{b}_{j}",
                        tag=f"ps{j}",
                    )
                    for k in range(KC):
                        xt, kk = chunk(k)
                        nc.tensor.matmul(
                            ps[:],
                            xt[:, 0, kk, 128 * j : 128 * (j + 1)],  # lhsT [d,n]
                            xt[:, 1, kk, :],                        # rhs  [d,m]
                            start=(k == 0),
                            stop=(k == KC - 1),
                        )
                    so = so_pool.tile(
                        [128, N], mybir.dt.bfloat16, name=f"so_{b}_{j}",
                        tag=f"so_{b}_{j}",
                    )
                    nc.vector.tensor_copy(so[:], ps[:])
                    so_tiles.append((b, j, so))

            # stores emitted after all loads so they never block a load's
            # issue in either HWDGE engine stream; alternate the two rings
            for i, (b, j, so) in enumerate(so_tiles):
                eng = nc.sync if i % 2 == 0 else nc.scalar
                eng.dma_start(out=sim[b, :, j, :], in_=so[:])

    nc.compile()
    _NC_CACHE["nc"] = nc
    return nc


def _l2norm(x):
    n = np.sqrt(np.sum(np.square(x), axis=-1, keepdims=True, dtype=np.float32))
    return x / np.maximum(n, np.float32(1e-12))


def _pack_inputs(sh, th):
    """(B,N,D) f32 x2 -> (B, 2, 128, 2, KC//2, N) bf16 p-major feature layout.

    x[b, h, p, st, kk, :] = feat_st[b, :, (h*KC//2 + kk)*128 + p] -- the
    (D, N)-transposed features with d split as (h, kk, p), p on partitions.
    """
    KH = KC // 2
    # (B, 2st, N, D) -> (B, 2st, N, 2h, KH, 128p) -> (B, 2h, 128p, 2st, KH, N)
    f = np.stack([sh, th], axis=1).reshape(B, 2, N, 2, KH, 128)
    return np.ascontiguousarray(f.transpose(0, 3, 5, 1, 4, 2)).astype(
        ml_dtypes.bfloat16
    )


def _hungarian_min(cost):
    """Exact square linear_sum_assignment (minimize); numpy fallback."""
    n = cost.shape[0]
    c = np.asarray(cost, dtype=np.float64)
    INF = np.inf
    u = np.zeros(n + 1)
    v = np.zeros(n + 1)
    p = np.zeros(n + 1, dtype=np.int64)
    way = np.zeros(n + 1, dtype=np.int64)
    for i in range(1, n + 1):
        p[0] = i
        j0 = 0
        minv = np.full(n + 1, INF)
        used = np.zeros(n + 1, dtype=bool)
        while True:
            used[j0] = True
            i0 = p[j0]
            cur = c[i0 - 1, :] - u[i0] - v[1:]
            free = ~used[1:]
            better = free & (cur < minv[1:])
            minv[1:][better] = cur[better]
            way[1:][better] = j0
            masked = np.where(free, minv[1:], INF)
            j1 = int(np.argmin(masked)) + 1
            delta = masked[j1 - 1]
            u[p[used]] += delta
            v[used] -= delta
            minv[1:][free] -= delta
            j0 = j1
            if p[j0] == 0:
                break
        while j0:
            j1 = way[j0]
            p[j0] = p[j1]
            j0 = j1
    col4row = np.zeros(n, dtype=np.int64)
    for j in range(1, n + 1):
        col4row[p[j] - 1] = j - 1
    return col4row


def _assign(sim_b):
    """col4row for maximizing sim_b (minimize -sim_b)."""
    try:
        from scipy.optimize import linear_sum_assignment

        r, c = linear_sum_assignment(-np.asarray(sim_b, dtype=np.float64))
        col = np.empty(sim_b.shape[0], dtype=np.int64)
        col[r] = c
        return col
    except ImportError:
        return _hungarian_min(-np.asarray(sim_b, dtype=np.float64))


def kernel(student_features, teacher_features):
    from concourse.bass_utils import run_bass_kernel_spmd

    s = np.asarray(student_features, dtype=np.float32)
    t = np.asarray(teacher_features, dtype=np.float32)

    sh = _l2norm(s)  # (B, N, D) f32
    th = _l2norm(t)

    x = _pack_inputs(sh, th)
    nc = _build_bass()
    in_maps = [{"x": x[c * BPC : (c + 1) * BPC]} for c in range(N_CORES)]
    res = run_bass_kernel_spmd(nc, in_maps, list(range(N_CORES)))
    # device sim layout: (BPC, p, j, m) with n = j*128 + p
    sim = np.concatenate(
        [np.asarray(res.results[c]["sim"]) for c in range(N_CORES)], axis=0
    )  # (B, 128, 2, N) bf16
    sim = sim.transpose(0, 2, 1, 3).reshape(B, N, N).astype(np.float32)

    # Hungarian on device sims; exact f32 gather of the matched cosine values.
    total = 0.0
    for b in range(B):
        col = _assign(sim[b])
        # matched[i] = <sh[b,i], th[b,col[i]]> recomputed exactly
        total += np.einsum(
            "nd,nd->n", sh[b].astype(np.float64), th[b][col].astype(np.float64)
        ).sum()
    loss = -(total / (B * N))
    return np.float32(loss)


# revision 17
# speedup vs baseline: 1.0754x; 1.0754x over previous
"""CrossMatchingLoss Trainium2 kernel.

Problem: loss = -mean(matched cosine sims) where the matching is an exact
Hungarian assignment per batch element (detached / solved on CPU, exactly as
the reference does).

Split of work:
  host : L2-normalize (f32), transpose features to (D, N) layout, cast bf16,
         shard batches 4-per-core across 8 NeuronCores.
  device (per core): sim[b] = s_hat[b] @ t_hat[b]^T for its 4 batches as
         bf16 matmuls with f32 PSUM accumulation -> bf16 sim matrices.
  host : Hungarian assignment per batch on the device sim, then gather the
         matched cosine values (recomputed in f32/f64 for exactness) and
         average.

Shapes are hardcoded for B=32, N=256, D=1024, 8 cores (4 batches per core).
"""

import numpy as np
import ml_dtypes

B, N, D = 32, 256, 1024
N_CORES = 8
BPC = B // N_CORES          # batches per core
KC = D // 128               # contraction k-chunks of 128

_NC_CACHE = {}


def _build_bass():
    """Build (once) the per-core Bass program: 4x [256,1024]@[1024,256]^T."""
    if "nc" in _NC_CACHE:
        return _NC_CACHE["nc"]

    import concourse.bacc as bacc
    import concourse.mybir as mybir
    import concourse.tile as tile

    nc = bacc.Bacc("TRN2")
    # p-major packing split in two k-halves: x[b, h, p, st, kk, :] =
    # feat_st[b, (h*4 + kk)*128 + p, :]. Each partition's data per DMA is one
    # contiguous 4KB run; the two halves load on different HWDGE rings.
    KH = KC // 2  # 4 k-chunks per half
    x = nc.dram_tensor(
        "x", [BPC, 2, 128, 2, KH, N], mybir.dt.bfloat16, kind="ExternalInput"
    )
    sim = nc.dram_tensor(
        "sim", [BPC, 128, 2, N], mybir.dt.bfloat16, kind="ExternalOutput"
    )

    with tile.TileContext(nc) as tc:
        with (
            tc.tile_pool(name="xin", bufs=4) as xin_pool,
            tc.tile_pool(name="so", bufs=1) as so_pool,
            tc.tile_pool(name="ps", bufs=2, space="PSUM") as ps_pool,
        ):
            # dummy matmuls during the initial DMA wait: get the PE past the
            # HAM cold window (~3.4us) so real matmuls run at 2.4 GHz
            zt = xin_pool.tile([128, 128], mybir.dt.bfloat16, name="zt", tag="zt")
            nc.gpsimd.memset(zt[:], 0.0)
            ps_w = ps_pool.tile(
                [128, 128], mybir.dt.float32, name="ps_w", tag="warm"
            )
            for _ in range(20):
                nc.tensor.matmul(ps_w[:], zt[:], zt[:], start=True, stop=True)

            so_tiles = []
            for b in range(BPC):
                xh0 = xin_pool.tile(
                    [128, 2, KH, N], mybir.dt.bfloat16, name=f"xh0_{b}", tag="h0"
                )
                nc.sync.dma_start(out=xh0[:], in_=x[b, 0])
                if b < BPC - 1:
                    xh1 = xin_pool.tile(
                        [128, 2, KH, N], mybir.dt.bfloat16, name=f"xh1_{b}",
                        tag="h1",
                    )
                    nc.scalar.dma_start(out=xh1[:], in_=x[b, 1])

                    def chunk(k, xh0=xh0, xh1=xh1):
                        return (xh0, k) if k < KH else (xh1, k - KH)
                else:
                    # last batch: teacher half as two quarter loads so only a
                    # couple of matmuls trail the final chunk's arrival
                    xq0 = xin_pool.tile(
                        [128, 2, 2, N], mybir.dt.bfloat16, name="xq0", tag="q0"
                    )
                    xq1 = xin_pool.tile(
                        [128, 2, 2, N], mybir.dt.bfloat16, name="xq1", tag="q1"
                    )
                    nc.scalar.dma_start(out=xq0[:], in_=x[b, 1][:, :, 0:2, :])
                    nc.scalar.dma_start(out=xq1[:], in_=x[b, 1][:, :, 2:4, :])

                    def chunk(k, xh0=xh0, xq0=xq0, xq1=xq1):
                        if k < KH:
                            return (xh0, k)
                        if k < KH + 2:
                            return (xq0, k - KH)
                        return (xq1, k - KH - 2)

                for j in range(2):
                    ps = ps_pool.tile(
                        [128, N], mybir.dt.float32, name=f"ps_{b}_{j}",
                        tag=f"ps{j}",
                    )
                    for k in range(KC):
                        xt, kk = chunk(k)
                        nc.tensor.matmul(
                            ps[:],
                            xt[:, 0, kk, 128 * j : 128 * (j + 1)],  # lhsT [d,n]
                            xt[:, 1, kk, :],                        # rhs  [d,m]
                            start=(k == 0),
                            stop=(k == KC - 1),
                        )
                    so = so_pool.tile(
                        [128, N], mybir.dt.bfloat16, name=f"so_{b}_{j}",
                        tag=f"so_{b}_{j}",
                    )
                    nc.vector.tensor_copy(so[:], ps[:])
                    so_tiles.append((b, j, so))

            # stores emitted after all loads so they never block a load's
            # issue in either HWDGE engine stream; alternate the two rings
            for i, (b, j, so) in enumerate(so_tiles):
                eng = nc.sync if i % 2 == 0 else nc.scalar
                eng.dma_start(out=sim[b, :, j, :], in_=so[:])

    nc.compile()
    _NC_CACHE["nc"] = nc
    return nc


def _l2norm(x):
    n = np.sqrt(np.sum(np.square(x), axis=-1, keepdims=True, dtype=np.float32))
    return x / np.maximum(n, np.float32(1e-12))


def _pack_inputs(sh, th):
    """(B,N,D) f32 x2 -> (B, 2, 128, 2, KC//2, N) bf16 p-major feature layout.

    x[b, h, p, st, kk, :] = feat_st[b, :, (h*KC//2 + kk)*128 + p] -- the
    (D, N)-transposed features with d split as (h, kk, p), p on partitions.
    """
    KH = KC // 2
    # (B, 2st, N, D) -> (B, 2st, N, 2h, KH, 128p) -> (B, 2h, 128p, 2st, KH, N)
    f = np.stack([sh, th], axis=1).reshape(B, 2, N, 2, KH, 128)
    return np.ascontiguousarray(f.transpose(0, 3, 5, 1, 4, 2)).astype(
        ml_dtypes.bfloat16
    )


def _hungarian_min(cost):
    """Exact square linear_sum_assignment (minimize); numpy fallback."""
    n = cost.shape[0]
    c = np.asarray(cost, dtype=np.float64)
    INF = np.inf
    u = np.zeros(n + 1)
    v = np.zeros(n + 1)
    p = np.zeros(n + 1, dtype=np.int64)
    way = np.zeros(n + 1, dtype=np.int64)
    for i in range(1, n + 1):
        p[0] = i
        j0 = 0
        minv = np.full(n + 1, INF)
        used = np.zeros(n + 1, dtype=bool)
        while True:
            used[j0] = True
            i0 = p[j0]
            cur = c[i0 - 1, :] - u[i0] - v[1:]
            free = ~used[1:]
            better = free & (cur < minv[1:])
            minv[1:][better] = cur[better]
            way[1:][better] = j0
            masked = np.where(free, minv[1:], INF)
            j1 = int(np.argmin(masked)) + 1
            delta = masked[j1 - 1]
            u[p[used]] += delta
            v[used] -= delta
            minv[1:][free] -= delta
            j0 = j1
            if p[j0] == 0:
                break
        while j0:
            j1 = way[j0]
            p[j0] = p[j1]
            j0 = j1
    col4row = np.zeros(n, dtype=np.int64)
    for j in range(1, n + 1):
        col4row[p[j] - 1] = j - 1
    return col4row


def _assign(sim_b):
    """col4row for maximizing sim_b (minimize -sim_b)."""
    try:
        from scipy.optimize import linear_sum_assignment

        r, c = linear_sum_assignment(-np.asarray(sim_b, dtype=np.float64))
        col = np.empty(sim_b.shape[0], dtype=np.int64)
        col[r] = c
        return col
    except ImportError:
        return _hungarian_min(-np.asarray(sim_b, dtype=np.float64))


def kernel(student_features, teacher_features):
    from concourse.bass_utils import run_bass_kernel_spmd

    s = np.asarray(student_features, dtype=np.float32)
    t = np.asarray(teacher_features, dtype=np.float32)

    sh = _l2norm(s)  # (B, N, D) f32
    th = _l2norm(t)

    x = _pack_inputs(sh, th)
    nc = _build_bass()
    in_maps = [{"x": x[c * BPC : (c + 1) * BPC]} for c in range(N_CORES)]
    res = run_bass_kernel_spmd(nc, in_maps, list(range(N_CORES)))
    # device sim layout: (BPC, p, j, m) with n = j*128 + p
    sim = np.concatenate(
        [np.asarray(res.results[c]["sim"]) for c in range(N_CORES)], axis=0
    )  # (B, 128, 2, N) bf16
    sim = sim.transpose(0, 2, 1, 3).reshape(B, N, N).astype(np.float32)

    # Hungarian on device sims; exact f32 gather of the matched cosine values.
    total = 0.0
    for b in range(B):
        col = _assign(sim[b])
        # matched[i] = <sh[b,i], th[b,col[i]]> recomputed exactly
        total += np.einsum(
            "nd,nd->n", sh[b].astype(np.float64), th[b][col].astype(np.float64)
        ).sum()
    loss = -(total / (B * N))
    return np.float32(loss)
